# revision 5
# baseline (speedup 1.0000x reference)
"""Axial attention (B,H,W,C)=(8,128,128,256), 8 heads, for 8 trn2 NeuronCores.

Sharding: data-parallel over batch B=8 -> one batch element per core.
Per core, two passes over x[b]:
  phase A: attention along H (one sequence per column w), writes
           oh + bout0 + bout1 to a bf16 HBM scratch in (H,W,C) layout.
  phase B: attention along W (one sequence per row h), adds the scratch row
           and writes the final fp32 output row.

Per-sequence math (t=128 tokens, C=256, 8 heads of e=32), all matmuls bf16:
  ST = S^T via PE transpose; QT/KT = W^T @ ST batched over 4 sequences;
  V per sequence with a fused ones-column per head so the attention
  denominator falls out of the AV matmul; scores computed transposed
  (softmax denominator = sum over partition j comes from the ones column),
  exp on the scalar engine with no max-subtraction (scores are O(1) by
  construction: Wq is pre-scaled by e^-0.5 on the host).
"""

import sys

sys.path.insert(0, "/opt/trn_rl_repo")

import numpy as np
import ml_dtypes

import concourse.bass as bass
import concourse.tile as tile
from concourse import mybir
from concourse.bass_utils import run_bass_kernel_spmd
from concourse.masks import make_identity
from concourse.vector_clock import ScopedClock

F32 = mybir.dt.float32
BF16 = mybir.dt.bfloat16
AF = mybir.ActivationFunctionType
OP = mybir.AluOpType

H = 128
W = 128
C = 256
HEADS = 8
E = C // HEADS  # 32
T = 128  # sequence length for both axes
G = 4  # sequences processed per group (batched projections)

# --- workaround: this toolchain's codegen accepts at most ONE sync-wait per
# instruction; redistribute extra waits onto preceding same-engine nops. ---

_MAXW = 1


def _patched_drain_and_barrier(self, tick_clock, wait_clock):
    probe = self.nc.sync.nop(nofuse=True)
    wait_clock.add_sem_waits(probe.ins, ScopedClock({None: tick_clock.global_clock}))
    conds = list(probe.ins.sync_info.on_wait)
    probe.ins.sync_info.on_wait = conds[:_MAXW]
    rest = conds[_MAXW:]
    while rest:
        n2 = self.nc.sync.nop(nofuse=True)
        if n2.ins.sync_info is None:
            n2.ins.sync_info = mybir.SyncInfo(on_wait=[], on_update=[])
        n2.ins.sync_info.on_wait = rest[:_MAXW]
        rest = rest[_MAXW:]
    self.nc.sync.drain()
    self.nc.all_engine_barrier()
    popped = self.nc._tile_sem_poison_stack.pop()
    assert popped is self._sem_poison
    self.nc.clear_and_free_semaphores(list(self.sems.allocated().values()))
    self.nc.all_engine_barrier()


tile.TileContext._drain_and_barrier = _patched_drain_and_barrier


def _split_waits(nc, limit=_MAXW):
    """Hoist extra sync-waits onto fresh nops directly before their owner."""
    n_split = 0
    for fn in nc.m.functions:
        for blk in fn.blocks:
            insts = blk.instructions
            out = []
            for inst in insts:
                si = inst.sync_info
                if si is not None and len(si.on_wait) > limit:
                    waits = list(si.on_wait)
                    extra, keep = waits[:-limit], waits[-limit:]
                    k = 0
                    while extra:
                        nop = mybir.InstNoOp(
                            name=f"{inst.name}-wsplit{k}",
                            engine=inst.engine,
                            bass_nofuse=True,
                            sync_info=mybir.SyncInfo(
                                on_wait=extra[:limit], on_update=[]
                            ),
                        )
                        nc.register_instruction(nop, overwrite=True)
                        out.append(nop)
                        extra = extra[limit:]
                        k += 1
                        n_split += 1
                    si.on_wait = keep
                out.append(inst)
            blk.instructions = out
    return n_split


def _bcast_rows(handle_ap, rows):
    """AP that broadcasts a 1D dram tensor across `rows` partitions."""
    return bass.AP(
        tensor=handle_ap.tensor,
        offset=handle_ap.offset,
        ap=[[0, rows]] + [list(p) for p in handle_ap.ap],
    )


def _free_bcast(ap, n):
    """Append a step-0 free dim of size n to an AP (within-partition bcast)."""
    return bass.AP(
        tensor=ap.tensor,
        offset=ap.offset,
        ap=[list(p) for p in ap.ap] + [[0, n]],
    )


def _build():
    nc = bass.Bass("TRN2", target_bir_lowering=False, debug=False)

    x = nc.dram_tensor("x", [H, W, C], F32, kind="ExternalInput")
    wqkv0 = nc.dram_tensor("wqkv0", [C, 3 * C], BF16, kind="ExternalInput")
    wout0 = nc.dram_tensor("wout0", [C, C], BF16, kind="ExternalInput")
    wqkv1 = nc.dram_tensor("wqkv1", [C, 3 * C], BF16, kind="ExternalInput")
    wout1 = nc.dram_tensor("wout1", [C, C], BF16, kind="ExternalInput")
    bsum = nc.dram_tensor("bsum", [C], F32, kind="ExternalInput")
    out = nc.dram_tensor("out", [H, W, C], F32, kind="ExternalOutput")
    scratch = nc.dram_tensor("ohs", [H, W, C], BF16)

    x_ap = x.ap()
    out_ap = out.ap()
    sc_ap = scratch.ap()
    KC = C // 128  # 2 contraction chunks

    with tile.TileContext(nc) as tc:
        with (
            tc.tile_pool(name="const", bufs=1) as const,
            tc.tile_pool(name="work", bufs=2) as work,
            tc.tile_pool(name="ps_big", bufs=4, space="PSUM") as ps_big,
            tc.tile_pool(name="ps_sm", bufs=4, space="PSUM") as ps_sm,
        ):
            # ---- constants ----
            ident = const.tile([128, 128], BF16, tag="ident")
            make_identity(nc, ident)
            bsum_sb = const.tile([128, C], F32, tag="bsum")
            nc.gpsimd.dma_start(out=bsum_sb, in_=_bcast_rows(bsum.ap(), 128))

            wqkv_sb = {}
            wout_sb = {}
            for ax, (wqkv_d, wout_d) in enumerate(
                [(wqkv0, wout0), (wqkv1, wout1)]
            ):
                wq3 = wqkv_d.ap().rearrange("(k p) n -> k p n", p=128)
                wo2 = wout_d.ap().rearrange("(k p) n -> k p n", p=128)
                for k in range(KC):
                    t_qkv = const.tile([128, 3 * C], BF16, tag=f"wqkv{ax}{k}")
                    nc.gpsimd.dma_start(out=t_qkv, in_=wq3[k])
                    wqkv_sb[ax, k] = t_qkv
                    t_o = const.tile([128, C], BF16, tag=f"wout{ax}{k}")
                    nc.gpsimd.dma_start(out=t_o, in_=wo2[k])
                    wout_sb[ax, k] = t_o

            def axial_pass(ax, n_groups=W // G):
                """ax=0: sequences along H (fixed w). ax=1: along W (fixed h)."""
                for grp in range(n_groups):
                    j0 = grp * G
                    if ax == 0:
                        src = x_ap[:, j0 : j0 + G, :]
                    else:
                        src = x_ap[j0 : j0 + G].rearrange("h w c -> w h c")

                    s4 = work.tile([128, G, C], F32, tag="s4")
                    nc.sync.dma_start(out=s4, in_=src)
                    sb4 = work.tile([128, G, C], BF16, tag="sb4")
                    nc.gpsimd.tensor_copy(out=sb4, in_=s4)

                    if ax == 1:
                        ohrow = work.tile([128, G, C], BF16, tag="ohrow")
                        nc.sync.dma_start(
                            out=ohrow,
                            in_=sc_ap[j0 : j0 + G].rearrange("h w c -> w h c"),
                        )
                        og = work.tile([128, G, C], F32, tag="og")
                    else:
                        og = work.tile([128, G, C], BF16, tag="oa")

                    # ---- S^T for the G sequences: stb[k][c, s*128+t] ----
                    stb = []
                    for k in range(KC):
                        stb_k = work.tile([128, G * T], BF16, tag=f"stb{k}")
                        for s in range(G):
                            tr = ps_sm.tile([128, 128], BF16, tag="pssm")
                            nc.tensor.transpose(
                                tr, sb4[:, s, k * 128 : (k + 1) * 128], ident
                            )
                            nc.vector.tensor_copy(
                                out=stb_k[:, s * T : (s + 1) * T], in_=tr
                            )
                        stb.append(stb_k)

                    # ---- QT / KT batched over the group ----
                    qtb = []
                    ktb = []
                    for which, dst in ((0, qtb), (1, ktb)):
                        for m in range(KC):
                            pp = ps_big.tile([128, G * T], F32, tag="psbig")
                            for k in range(KC):
                                lhs = wqkv_sb[ax, k][
                                    :, which * C + m * 128 : which * C + (m + 1) * 128
                                ]
                                nc.tensor.matmul(
                                    pp,
                                    lhs,
                                    stb[k],
                                    start=(k == 0),
                                    stop=(k == KC - 1),
                                )
                            sb = work.tile(
                                [128, G * T], BF16, tag=f"qk{which}{m}"
                            )
                            nc.vector.tensor_copy(out=sb, in_=pp)
                            dst.append(sb)

                    for s in range(G):
                        # ---- V with fused ones column per head ----
                        vps = ps_big.tile([128, C], F32, tag="psbig")
                        for k in range(KC):
                            nc.tensor.matmul(
                                vps,
                                stb[k][:, s * T : (s + 1) * T],
                                wqkv_sb[ax, k][:, 2 * C : 3 * C],
                                start=(k == 0),
                                stop=(k == KC - 1),
                            )
                        vp = work.tile([128, HEADS * (E + 1)], BF16, tag="vp")
                        nc.gpsimd.memset(vp, 1.0)
                        vp3 = vp.rearrange("p (h q) -> p h q", q=E + 1)
                        nc.vector.tensor_copy(
                            out=vp3[:, :, 0:E],
                            in_=vps.rearrange("p (h e) -> p h e", e=E),
                        )

                        # ---- attention ----
                        ops = ps_big.tile([128, HEADS * (E + 1)], F32, tag="psbig")
                        for h in range(HEADS):
                            kq = h // 4
                            off = (h % 4) * E
                            scps = ps_sm.tile([128, 128], F32, tag="pssm")
                            nc.tensor.matmul(
                                scps,
                                ktb[kq][off : off + E, s * T : (s + 1) * T],
                                qtb[kq][off : off + E, s * T : (s + 1) * T],
                                start=True,
                                stop=True,
                                tile_position=(off, 0),
                            )
                            eb = work.tile([128, 128], BF16, tag="eb")
                            nc.scalar.activation(out=eb, in_=scps, func=AF.Exp)
                            nc.tensor.matmul(
                                ops[:, h * (E + 1) : (h + 1) * (E + 1)],
                                eb,
                                vp[:, h * (E + 1) : (h + 1) * (E + 1)],
                                start=True,
                                stop=True,
                            )

                        o3 = ops.rearrange("p (h q) -> p h q", q=E + 1)
                        recip = work.tile([128, HEADS], F32, tag="recip")
                        nc.vector.reciprocal(out=recip, in_=o3[:, :, E])
                        onorm = work.tile([128, C], BF16, tag="onorm")
                        nc.vector.tensor_tensor(
                            out=onorm.rearrange("p (h e) -> p h e", e=E),
                            in0=o3[:, :, 0:E],
                            in1=_free_bcast(recip[:], E),
                            op=OP.mult,
                        )

                        # ---- out projection ----
                        otb = work.tile([128, C], BF16, tag="otb")
                        for k in range(KC):
                            otps = ps_sm.tile([128, 128], BF16, tag="pssm")
                            nc.tensor.transpose(
                                otps, onorm[:, k * 128 : (k + 1) * 128], ident
                            )
                            nc.vector.tensor_copy(
                                out=otb[:, k * 128 : (k + 1) * 128], in_=otps
                            )
                        fps = ps_big.tile([128, C], F32, tag="psbig")
                        for k in range(KC):
                            nc.tensor.matmul(
                                fps,
                                otb[:, k * 128 : (k + 1) * 128],
                                wout_sb[ax, k],
                                start=(k == 0),
                                stop=(k == KC - 1),
                            )
                        if ax == 0:
                            nc.vector.tensor_tensor(
                                out=og[:, s, :], in0=fps, in1=bsum_sb, op=OP.add
                            )
                        else:
                            nc.vector.tensor_tensor(
                                out=og[:, s, :], in0=fps, in1=ohrow[:, s, :], op=OP.add
                            )

                    if ax == 0:
                        nc.sync.dma_start(out=sc_ap[:, j0 : j0 + G, :], in_=og)
                    else:
                        nc.sync.dma_start(
                            out=out_ap[j0 : j0 + G].rearrange("h w c -> w h c"),
                            in_=og,
                        )

            axial_pass(0)
            axial_pass(1)

    _split_waits(nc)
    return nc


_NC = None


def _get_nc():
    global _NC
    if _NC is None:
        _NC = _build()
    return _NC


def make_in_maps(x, Wq0, Wkv0, Wout0, bout0, Wq1, Wkv1, Wout1, bout1):
    bf = ml_dtypes.bfloat16
    scale = float(E) ** -0.5
    wqkv0 = np.concatenate([Wq0 * scale, Wkv0], axis=1).astype(bf)
    wqkv1 = np.concatenate([Wq1 * scale, Wkv1], axis=1).astype(bf)
    shared = {
        "wqkv0": wqkv0,
        "wout0": np.asarray(Wout0, dtype=bf),
        "wqkv1": wqkv1,
        "wout1": np.asarray(Wout1, dtype=bf),
        "bsum": np.asarray(bout0 + bout1, dtype=np.float32),
    }
    return [
        {"x": np.ascontiguousarray(x[b], dtype=np.float32), **shared}
        for b in range(x.shape[0])
    ]


def kernel(x, Wq0, Wkv0, Wout0, bout0, Wq1, Wkv1, Wout1, bout1):
    nc = _get_nc()
    in_maps = make_in_maps(
        np.asarray(x),
        np.asarray(Wq0),
        np.asarray(Wkv0),
        np.asarray(Wout0),
        np.asarray(bout0, dtype=np.float32),
        np.asarray(Wq1),
        np.asarray(Wkv1),
        np.asarray(Wout1),
        np.asarray(bout1, dtype=np.float32),
    )
    res = run_bass_kernel_spmd(nc, in_maps, core_ids=list(range(8)))
    return np.stack([r["out"] for r in res.results]).astype(np.float32)


# revision 23
# speedup vs baseline: 1.1512x; 1.1512x over previous
"""Axial attention (B,H,W,C)=(8,128,128,256), 8 heads, for 8 trn2 NeuronCores.

Sharding: data-parallel over batch B=8 -> one batch element per core.
Per core, two passes over x[b] (x pre-cast to bf16 on the host):
  phase A: attention along H (one sequence per column w), writes
           oh + bout0 + bout1 to a bf16 HBM scratch in (H,W,C) layout.
  phase B: attention along W (one sequence per row h), adds the scratch row
           and writes the final fp32 output row.

Per-sequence math (t=128 tokens, C=256, 8 heads of e=32), all matmuls bf16:
  S^T via XBAR dma-transpose (SBUF->SBUF, no PE/PSUM involved);
  QT/KT = W^T @ ST batched over 4 sequences; V per sequence with a fused
  ones-column per head so the attention denominator falls out of the AV
  matmul; scores computed transposed, 4 heads batched into one PSUM tile so
  a single [128,512] exp on the scalar engine covers them; no
  max-subtraction (scores are O(1): Wq is pre-scaled by e^-0.5 on the host).
"""

import sys

sys.path.insert(0, "/opt/trn_rl_repo")

import numpy as np
import ml_dtypes

import concourse.bass as bass
import concourse.tile as tile
from concourse import mybir
from concourse.bass_utils import run_bass_kernel_spmd
from concourse.vector_clock import ScopedClock

F32 = mybir.dt.float32
BF16 = mybir.dt.bfloat16
AF = mybir.ActivationFunctionType
OP = mybir.AluOpType

H = 128
W = 128
C = 256
HEADS = 8
E = C // HEADS  # 32
T = 128  # sequence length for both axes
G = 4  # sequences processed per group (batched projections)

# --- workaround: this toolchain's codegen accepts at most ONE sync-wait per
# instruction; redistribute extra waits onto preceding same-engine nops. ---

_MAXW = 1


def _patched_drain_and_barrier(self, tick_clock, wait_clock):
    probe = self.nc.sync.nop(nofuse=True)
    wait_clock.add_sem_waits(probe.ins, ScopedClock({None: tick_clock.global_clock}))
    conds = list(probe.ins.sync_info.on_wait)
    probe.ins.sync_info.on_wait = conds[:_MAXW]
    rest = conds[_MAXW:]
    while rest:
        n2 = self.nc.sync.nop(nofuse=True)
        if n2.ins.sync_info is None:
            n2.ins.sync_info = mybir.SyncInfo(on_wait=[], on_update=[])
        n2.ins.sync_info.on_wait = rest[:_MAXW]
        rest = rest[_MAXW:]
    self.nc.sync.drain()
    self.nc.all_engine_barrier()
    popped = self.nc._tile_sem_poison_stack.pop()
    assert popped is self._sem_poison
    self.nc.clear_and_free_semaphores(list(self.sems.allocated().values()))
    self.nc.all_engine_barrier()


tile.TileContext._drain_and_barrier = _patched_drain_and_barrier


def _split_waits(nc, limit=_MAXW):
    """Hoist extra sync-waits onto fresh nops directly before their owner."""
    n_split = 0
    for fn in nc.m.functions:
        for blk in fn.blocks:
            insts = blk.instructions
            out = []
            for inst in insts:
                si = inst.sync_info
                if si is not None and len(si.on_wait) > limit:
                    waits = list(si.on_wait)
                    extra, keep = waits[:-limit], waits[-limit:]
                    k = 0
                    while extra:
                        nop = mybir.InstNoOp(
                            name=f"{inst.name}-wsplit{k}",
                            engine=inst.engine,
                            bass_nofuse=True,
                            sync_info=mybir.SyncInfo(
                                on_wait=extra[:limit], on_update=[]
                            ),
                        )
                        nc.register_instruction(nop, overwrite=True)
                        out.append(nop)
                        extra = extra[limit:]
                        k += 1
                        n_split += 1
                    si.on_wait = keep
                out.append(inst)
            blk.instructions = out
    return n_split


def _bcast_rows(handle_ap, rows):
    """AP that broadcasts a 1D dram tensor across `rows` partitions."""
    return bass.AP(
        tensor=handle_ap.tensor,
        offset=handle_ap.offset,
        ap=[[0, rows]] + [list(p) for p in handle_ap.ap],
    )


def _free_bcast(ap, n):
    """Append a step-0 free dim of size n to an AP (within-partition bcast)."""
    return bass.AP(
        tensor=ap.tensor,
        offset=ap.offset,
        ap=[list(p) for p in ap.ap] + [[0, n]],
    )


def _build():
    nc = bass.Bass("TRN2", target_bir_lowering=False, debug=False)

    # host-pre-transposed inputs: [group, c, s*128+t] so S^T tiles DMA directly
    xta = nc.dram_tensor("xta", [W // G, C, G * T], BF16, kind="ExternalInput")
    xtc = nc.dram_tensor("xtc", [H // G, C, G * T], BF16, kind="ExternalInput")
    wqkv0 = nc.dram_tensor("wqkv0", [C, 3 * C], BF16, kind="ExternalInput")
    wout0 = nc.dram_tensor("wout0", [C, C], BF16, kind="ExternalInput")
    wqkv1 = nc.dram_tensor("wqkv1", [C, 3 * C], BF16, kind="ExternalInput")
    wout1 = nc.dram_tensor("wout1", [C, C], BF16, kind="ExternalInput")
    bsum = nc.dram_tensor("bsum", [C], F32, kind="ExternalInput")
    out = nc.dram_tensor("out", [H, W, C], F32, kind="ExternalOutput")
    scratch = nc.dram_tensor("ohs", [H, W, C], BF16)

    xta_ap = xta.ap()
    xtc_ap = xtc.ap()
    out_ap = out.ap()
    sc_ap = scratch.ap()
    KC = C // 128  # 2 contraction chunks

    with tile.TileContext(nc) as tc:
        with (
            tc.tile_pool(name="const", bufs=1) as const,
            tc.tile_pool(name="work", bufs=3) as work,
            tc.tile_pool(name="ps", bufs=8, space="PSUM") as ps,
        ):
            # ---- constants ----
            ident = const.tile([128, 128], BF16, tag="ident")
            from concourse.masks import make_identity

            make_identity(nc, ident)
            bsum_sb = const.tile([128, C], F32, tag="bsum")
            nc.gpsimd.dma_start(out=bsum_sb, in_=_bcast_rows(bsum.ap(), 128))

            wqkv_sb = {}
            wout_sb = {}
            for ax, (wqkv_d, wout_d) in enumerate(
                [(wqkv0, wout0), (wqkv1, wout1)]
            ):
                wq3 = wqkv_d.ap().rearrange("(k p) n -> k p n", p=128)
                wo2 = wout_d.ap().rearrange("(k p) n -> k p n", p=128)
                for k in range(KC):
                    t_qkv = const.tile([128, 3 * C], BF16, tag=f"wqkv{ax}{k}")
                    nc.gpsimd.dma_start(out=t_qkv, in_=wq3[k])
                    wqkv_sb[ax, k] = t_qkv
                    t_o = const.tile([128, C], BF16, tag=f"wout{ax}{k}")
                    nc.gpsimd.dma_start(out=t_o, in_=wo2[k])
                    wout_sb[ax, k] = t_o

            # persistent V'-buffers: ones columns written once, V columns
            # overwritten every sequence
            NVP = 8
            vp_bufs = []
            for i in range(NVP):
                vpb = const.tile([128, HEADS * (E + 1)], BF16, tag=f"vp{i}")
                nc.gpsimd.memset(vpb, 1.0)
                vp_bufs.append(vpb)

            def axial_pass(ax, n_groups=W // G):
                """ax=0: sequences along H (fixed w). ax=1: along W (fixed h)."""
                for grp in range(n_groups):
                    j0 = grp * G
                    xt_ap = xta_ap if ax == 0 else xtc_ap

                    if ax == 1:
                        ohrow = work.tile([128, G, C], BF16, tag="ohrow")
                        nc.sync.dma_start(
                            out=ohrow,
                            in_=sc_ap[j0 : j0 + G].rearrange("h w c -> w h c"),
                        )
                        og = work.tile([128, G, C], F32, tag="og")
                    else:
                        og = work.tile([128, G, C], BF16, tag="oa")

                    # ---- S^T loads directly (host pre-transposed) ----
                    stb = []
                    for k in range(KC):
                        stb_k = work.tile([128, G * T], BF16, tag=f"stb{k}")
                        nc.sync.dma_start(
                            out=stb_k, in_=xt_ap[grp, k * 128 : (k + 1) * 128, :]
                        )
                        stb.append(stb_k)

                    # ---- QT / KT batched over the group ----
                    qtb = []
                    ktb = []
                    for which, dst in ((0, qtb), (1, ktb)):
                        for m in range(KC):
                            pp = ps.tile([128, G * T], F32, tag="ps")
                            for k in range(KC):
                                lhs = wqkv_sb[ax, k][
                                    :, which * C + m * 128 : which * C + (m + 1) * 128
                                ]
                                nc.tensor.matmul(
                                    pp,
                                    lhs,
                                    stb[k],
                                    start=(k == 0),
                                    stop=(k == KC - 1),
                                )
                            sb = work.tile(
                                [128, G * T], BF16, tag=f"qk{which}{m}"
                            )
                            nc.vector.tensor_copy(out=sb, in_=pp)
                            dst.append(sb)

                    # ---- stage-major over the G sequences: every stage emits
                    # all 4 sequences' ops back-to-back so the in-order
                    # engine queues pipeline across sequences ----

                    # V (fused ones column per head)
                    vps_l = []
                    for s in range(G):
                        vps = ps.tile([128, C], F32, tag="ps")
                        for k in range(KC):
                            nc.tensor.matmul(
                                vps,
                                stb[k][:, s * T : (s + 1) * T],
                                wqkv_sb[ax, k][:, 2 * C : 3 * C],
                                start=(k == 0),
                                stop=(k == KC - 1),
                            )
                        vps_l.append(vps)
                    vp_l = []
                    for s in range(G):
                        vp = vp_bufs[(grp * G + s) % NVP]
                        vp3 = vp.rearrange("p (h q) -> p h q", q=E + 1)
                        nc.vector.tensor_copy(
                            out=vp3[:, :, 0:E],
                            in_=vps_l[s].rearrange("p (h e) -> p h e", e=E),
                        )
                        vp_l.append(vp)

                    # attention, 4 heads (one chunk) at a time. Scores are
                    # batched per tile-position q ACROSS the 4 sequences: the
                    # 4 matmuls into one PSUM tile share one PE sub-array
                    # (same tile_position) so they serialize naturally —
                    # concurrent row-tiles never touch the same PSUM bank.
                    ops_l = [None] * G
                    for hg in range(2):
                        scq_l = []
                        for q in range(4):
                            off = q * E
                            scq = ps.tile([128, G * T], F32, tag="ps")
                            for s in range(G):
                                nc.tensor.matmul(
                                    scq[:, s * T : (s + 1) * T],
                                    ktb[hg][off : off + E, s * T : (s + 1) * T],
                                    qtb[hg][off : off + E, s * T : (s + 1) * T],
                                    start=True,
                                    stop=True,
                                    tile_position=(off, 0),
                                )
                            scq_l.append(scq)
                        ebq_l = []
                        for q in range(4):
                            ebq = work.tile([128, G * T], BF16, tag="eb4")
                            nc.scalar.activation(
                                out=ebq, in_=scq_l[q], func=AF.Exp
                            )
                            ebq_l.append(ebq)
                        for s in range(G):
                            if ops_l[s] is None:
                                ops_t = ps.tile(
                                    [128, HEADS * (E + 1)], F32, tag="ps"
                                )
                                ops_l[s] = ops_t
                            for q in range(4):
                                h = hg * 4 + q
                                nc.tensor.matmul(
                                    ops_l[s][:, h * (E + 1) : (h + 1) * (E + 1)],
                                    ebq_l[q][:, s * T : (s + 1) * T],
                                    vp_l[s][:, h * (E + 1) : (h + 1) * (E + 1)],
                                    start=True,
                                    stop=True,
                                )

                    # normalize (divide by the fused denominator column)
                    recip_l = []
                    for s in range(G):
                        o3 = ops_l[s].rearrange("p (h q) -> p h q", q=E + 1)
                        recip = work.tile([128, HEADS], F32, tag="recip")
                        nc.vector.reciprocal(out=recip, in_=o3[:, :, E])
                        recip_l.append(recip)
                    onorm_l = []
                    for s in range(G):
                        o3 = ops_l[s].rearrange("p (h q) -> p h q", q=E + 1)
                        onorm = work.tile([128, C], BF16, tag="onorm")
                        nc.vector.tensor_tensor(
                            out=onorm.rearrange("p (h e) -> p h e", e=E),
                            in0=o3[:, :, 0:E],
                            in1=_free_bcast(recip_l[s][:], E),
                            op=OP.mult,
                        )
                        onorm_l.append(onorm)

                    # out projection (OT via batched PE transpose)
                    otb_l = []
                    for s in range(G):
                        ot_ps = ps.tile([128, C], BF16, tag="ps")
                        for k in range(KC):
                            nc.tensor.transpose(
                                ot_ps[:, k * 128 : (k + 1) * 128],
                                onorm_l[s][:, k * 128 : (k + 1) * 128],
                                ident,
                            )
                        otb = work.tile([128, C], BF16, tag="otb")
                        nc.vector.tensor_copy(out=otb, in_=ot_ps)
                        otb_l.append(otb)
                    fps_l = []
                    for s in range(G):
                        fps = ps.tile([128, C], F32, tag="ps")
                        for k in range(KC):
                            nc.tensor.matmul(
                                fps,
                                otb_l[s][:, k * 128 : (k + 1) * 128],
                                wout_sb[ax, k],
                                start=(k == 0),
                                stop=(k == KC - 1),
                            )
                        fps_l.append(fps)
                    for s in range(G):
                        if ax == 0:
                            nc.vector.tensor_tensor(
                                out=og[:, s, :], in0=fps_l[s], in1=bsum_sb,
                                op=OP.add,
                            )
                        else:
                            nc.vector.tensor_tensor(
                                out=og[:, s, :], in0=fps_l[s],
                                in1=ohrow[:, s, :], op=OP.add,
                            )

                    if ax == 0:
                        nc.sync.dma_start(out=sc_ap[:, j0 : j0 + G, :], in_=og)
                    else:
                        nc.sync.dma_start(
                            out=out_ap[j0 : j0 + G].rearrange("h w c -> w h c"),
                            in_=og,
                        )

            axial_pass(0)
            axial_pass(1)

    _split_waits(nc)
    return nc


_NC = None


def _get_nc():
    global _NC
    if _NC is None:
        _NC = _build()
    return _NC


def make_in_maps(x, Wq0, Wkv0, Wout0, bout0, Wq1, Wkv1, Wout1, bout1):
    bf = ml_dtypes.bfloat16
    scale = float(E) ** -0.5
    wqkv0 = np.concatenate([Wq0 * scale, Wkv0], axis=1).astype(bf)
    wqkv1 = np.concatenate([Wq1 * scale, Wkv1], axis=1).astype(bf)
    xb = np.asarray(x, dtype=bf)
    shared = {
        "wqkv0": wqkv0,
        "wout0": np.asarray(Wout0, dtype=bf),
        "wqkv1": wqkv1,
        "wout1": np.asarray(Wout1, dtype=bf),
        "bsum": np.asarray(bout0 + bout1, dtype=np.float32),
    }
    maps = []
    for b in range(x.shape[0]):
        e = xb[b]  # (H, W, C)
        # xta[g, c, s*T+h] = x[h, 4g+s, c]  (phase A: sequences along H)
        xta_b = np.ascontiguousarray(
            e.transpose(1, 2, 0).reshape(W // G, G, C, H).transpose(0, 2, 1, 3)
        ).reshape(W // G, C, G * T)
        # xtc[g, c, s*T+w] = x[4g+s, w, c]  (phase B: sequences along W)
        xtc_b = np.ascontiguousarray(
            e.reshape(H // G, G, W, C).transpose(0, 3, 1, 2)
        ).reshape(H // G, C, G * T)
        maps.append({"xta": xta_b, "xtc": xtc_b, **shared})
    return maps


def kernel(x, Wq0, Wkv0, Wout0, bout0, Wq1, Wkv1, Wout1, bout1):
    nc = _get_nc()
    in_maps = make_in_maps(
        np.asarray(x),
        np.asarray(Wq0),
        np.asarray(Wkv0),
        np.asarray(Wout0),
        np.asarray(bout0, dtype=np.float32),
        np.asarray(Wq1),
        np.asarray(Wkv1),
        np.asarray(Wout1),
        np.asarray(bout1, dtype=np.float32),
    )
    res = run_bass_kernel_spmd(nc, in_maps, core_ids=list(range(8)))
    return np.stack([r["out"] for r in res.results]).astype(np.float32)
